# revision 28
# baseline (speedup 1.0000x reference)
"""Dilated KNN (k=9, dilation=2) over query[4, 8192, 64] on 8 NeuronCores.

Sharding: batch b and query-half h per core (core = 2*b + h). Each core
computes scores s[m, n] = 2*x_m.x_n - |x_n|^2 for its 4096 queries against
all 8192 supports of its batch, then selects the top-17 per row and emits
indices of ranks 0, 2, ..., 16.

Selection pipeline (single-scan, index-packed):
  PE  : fp32r hi/lo split matmul -> PSUM scores (exact products)
  ACT : u = uint32(Relu(-alpha2*s + 1.5*2^31)) -- the [2^31, 2^32) binade
        has ulp 256, so the cast's low 8 bits are zero for free; smaller u
        means better score
  POOL: p = u + n8 (n8 = column mod 256) -- packs the chunk-local index
        into the zeroed low byte; ties break toward the lowest index.
        A tunable fraction of spans packs on DVE (bitwise_or) instead to
        balance engine load.
  DVE : one MAX8 per 256-chunk on p bitcast to f32 (negative-float order
        reverses, so max8 finds the smallest u = best scores), then a
        3-round merge over the 256 candidates. Winner packed values and
        candidate positions DMA out; the host decodes
        idx = (pos>>3)*256 + (val&255). No second scan, no gathers.

Setup is kept off the ACT engine (DVE copies + DMA) so the main-loop
pipeline starts as early as possible; tile 0's query prep is emitted
before the support-side prep.
"""

import sys
import types

import numpy as np

B = 4
N = 8192
C = 64
K_OUT = 9
NQ = N // 2
N_CORES = 8
PCHUNK = 512              # psum matmul chunk (columns per bank)
SCHUNK = 256              # selection chunk (8-bit local index)
SPAN = 4096               # ACT/pool span (8 psum chunks, 16 sel chunks)
N_SPANS = N // SPAN       # 4
DVE_PACK_EVERY = 10 ** 9  # DVE int packs are slow/wrong; pool does all
ALPHA2 = 16000.0 * 256.0  # score quantization: delta = 256/ALPHA2 = 6.25e-5
OFFB = 1.5 * 2.0 ** 31
NEG_BIG = -1.0e38


def _install_ntff_shim():
    """bass_utils imports antenv.axon_hooks for trace=True; the agent image
    lacks it. Register the ctypes-based hook so NTFF profiling works."""
    if "antenv.axon_hooks" in sys.modules:
        return
    try:
        from trn_agent_boot.trn_boot import _ntff_profile_via_ctypes

        hook = _ntff_profile_via_ctypes("/opt/axon/libaxon_pjrt.so")
        m = types.ModuleType("antenv.axon_hooks")
        m.get_axon_ntff_profile_hook = lambda: hook
        sys.modules["antenv.axon_hooks"] = m
    except Exception:
        pass


def build_kernel(nc, n_queries=NQ):
    import concourse.mybir as mybir
    import concourse.tile as tile
    from concourse import masks

    F32 = mybir.dt.float32
    F32R = mybir.dt.float32r
    U32 = mybir.dt.uint32
    I32 = mybir.dt.int32

    m_tiles = n_queries // 128
    xq = nc.dram_tensor("xq", [n_queries, C], F32, kind="ExternalInput")
    xs = nc.dram_tensor("xs", [N, C], F32, kind="ExternalInput")
    # packed winner values and candidate positions; host decodes
    # idx = (pos >> 3) * 256 + (val & 255)
    outv = nc.dram_tensor("pkv", [n_queries, K_OUT], I32, kind="ExternalOutput")
    outp = nc.dram_tensor("pkp", [n_queries, K_OUT], I32, kind="ExternalOutput")
    sqh_d = nc.dram_tensor("sqh_d", [C, 128], F32, kind="Internal")
    sql_d = nc.dram_tensor("sql_d", [C, 128], F32, kind="Internal")

    with tile.TileContext(nc) as tc:
        with (
            tc.tile_pool(name="const", bufs=1) as constp,
            tc.tile_pool(name="big", bufs=1) as bigp,
        ):
            identity = constp.tile([128, 128], F32)
            masks.make_identity(nc, identity[:, :])
            ones2 = constp.tile([2, PCHUNK], F32)
            nc.vector.memset(ones2[:, :], 1.0)

            # n8 span constant: col mod 256, as uint32
            n8u = constp.tile([128, SPAN], U32)
            with tc.tile_pool(name="n8tmp", bufs=1) as n8tmpp:
                n8f = n8tmpp.tile([128, SPAN], F32)
                nc.gpsimd.iota(
                    n8f[:, :],
                    pattern=[[0, SPAN // SCHUNK], [1, SCHUNK]],
                    base=0,
                    channel_multiplier=0,
                    allow_small_or_imprecise_dtypes=True,
                )
                nc.vector.tensor_copy(n8u[:, :], n8f[:, :])

            offb = constp.tile([128, 1], F32)
            nc.vector.memset(offb[:, :], OFFB)

            sqall = bigp.tile([128, C], F32)  # sq per support, [row, chunkblk]
            rhs1 = bigp.tile([128, N], F32R)
            rhs2 = bigp.tile([66, N], F32R)
            lhsT1 = bigp.tile([128, n_queries], F32R)
            lhsT2 = bigp.tile([66, n_queries], F32R)

            # lhsT2 ones rows don't depend on inputs
            nc.sync.dma_start(
                lhsT2[64:66, :].bitcast(F32).rearrange("p (r c) -> p r c", c=PCHUNK),
                ones2[:, :].unsqueeze(1).broadcast_to(
                    [2, n_queries // PCHUNK, PCHUNK]
                ),
            )

            with (
                tc.tile_pool(name="stage", bufs=4) as stagep,
                tc.tile_pool(name="dtmp", bufs=4) as dtmp,
                tc.tile_pool(name="ptr", bufs=4, space="PSUM") as ptrp,
                tc.tile_pool(name="psq", bufs=2, space="PSUM") as psqp,
            ):

                # query side first (engine queues drain these fast), so
                # the per-engine monotonic semaphores don't couple the main
                # loop to late setup ops; staged 4 tiles per DMA to keep the
                # sync queue short
                for q4 in range(m_tiles // 4):
                    st4 = stagep.tile([128, 4, C], F32, tag="stq")
                    nc.sync.dma_start(
                        st4[:, :, :],
                        xq.ap()[q4 * 512 : (q4 + 1) * 512, :].rearrange(
                            "(k p) c -> p k c", p=128
                        ),
                    )
                    for k in range(4):
                        t = q4 * 4 + k
                        qsl = slice(t * 128, (t + 1) * 128)
                        pt = ptrp.tile([C, 128], F32)
                        nc.tensor.transpose(
                            pt[:, :], st4[:, k, :], identity[:, :]
                        )
                        nc.scalar.mul(lhsT1[0:64, qsl], pt[:, :], 2.0)  # 2ah
                        # 2al = 2a - 2ah in one DVE op (F32R write rounds)
                        nc.vector.scalar_tensor_tensor(
                            lhsT1[64:128, qsl],
                            pt[:, :],
                            2.0,
                            lhsT1[0:64, qsl].bitcast(F32),
                            mybir.AluOpType.mult,
                            mybir.AluOpType.subtract,
                        )
                nc.sync.dma_start(
                    lhsT2[0:64, :].bitcast(F32), lhsT1[0:64, :].bitcast(F32)
                )

                # support side
                for cc in range(N // PCHUNK):
                    sl = slice(cc * PCHUNK, (cc + 1) * PCHUNK)
                    sqcol = sqall[:, cc * 4 : (cc + 1) * 4]
                    st4 = stagep.tile([128, 4, C], F32, tag="sts")
                    nc.sync.dma_start(
                        st4[:, :, :],
                        xs.ap()[sl, :].rearrange("(k p) c -> p k c", p=128),
                    )
                    sqscr = dtmp.tile([128, 4 * C], F32, tag="sqscr")
                    # |x_n|^2 per support row while it's still [n, c]
                    # (tensor_tensor_reduce hangs TRN2 here; use mul+reduce)
                    nc.vector.tensor_mul(
                        sqscr[:, :].rearrange("p (k c) -> p k c", c=C),
                        st4[:, :, :],
                        st4[:, :, :],
                    )
                    nc.vector.reduce_sum(
                        sqcol,
                        sqscr[:, :].rearrange("p (k c) -> p k c", c=C),
                        axis=mybir.AxisListType.X,
                    )
                    for k in range(PCHUNK // 128):
                        j = cc * (PCHUNK // 128) + k
                        jsl = slice(j * 128, (j + 1) * 128)
                        pt = ptrp.tile([C, 128], F32)
                        nc.tensor.transpose(
                            pt[:, :], st4[:, k, :], identity[:, :]
                        )
                        if k % 2 == 0:
                            nc.scalar.copy(rhs1[0:64, jsl], pt[:, :])  # bh
                        else:
                            nc.vector.tensor_copy(rhs1[0:64, jsl], pt[:, :])
                        nc.vector.tensor_sub(
                            rhs2[0:64, jsl], pt[:, :],
                            rhs1[0:64, jsl].bitcast(F32),
                        )  # bl
                    ptq = psqp.tile([PCHUNK // 128, 128], F32)
                    nc.tensor.transpose(ptq[:, :], sqcol, identity[:, :])
                    sq4 = dtmp.tile([PCHUNK // 128, 128], F32, tag="sq4")
                    nc.scalar.copy(sq4[:, :], ptq[:, :])
                    sqr = dtmp.tile([1, PCHUNK], F32, tag="sqr")
                    for k in range(PCHUNK // 128):
                        nc.sync.dma_start(
                            sqr[0:1, k * 128 : (k + 1) * 128], sq4[k : k + 1, :]
                        )
                    nsqh = dtmp.tile([1, PCHUNK], F32R, tag="nsqh")
                    nc.scalar.mul(nsqh[:, :], sqr[:, :], -1.0)  # -sqh
                    nc.sync.dma_start(rhs2[64:65, sl], nsqh[:, :])
                    sql = dtmp.tile([1, PCHUNK], F32, tag="sql")
                    nc.vector.tensor_add(sql[:, :], sqr[:, :], nsqh[:, :].bitcast(F32))
                    nsql = dtmp.tile([1, PCHUNK], F32R, tag="nsql")
                    nc.scalar.mul(nsql[:, :], sql[:, :], -1.0)  # -sql
                    nc.sync.dma_start(rhs2[65:66, sl], nsql[:, :])
                    nc.sync.dma_start(
                        rhs1[64:128, sl].bitcast(F32), rhs1[0:64, sl].bitcast(F32)
                    )

            with (
                tc.tile_pool(name="upool", bufs=3) as upool,
                tc.tile_pool(name="cpool", bufs=2) as cpool,
                tc.tile_pool(name="pmm", bufs=8, space="PSUM") as pmm,
            ):
                for t in range(m_tiles):
                    qsl = slice(t * 128, (t + 1) * 128)
                    cand = cpool.tile([128, 256], F32, tag="cand")
                    for j in range(N_SPANS):
                        u = upool.tile([128, SPAN], U32, tag="u")
                        for k in range(SPAN // PCHUNK):
                            cc = j * (SPAN // PCHUNK) + k
                            sl = slice(cc * PCHUNK, (cc + 1) * PCHUNK)
                            pm = pmm.tile([128, PCHUNK], F32, tag="pm")
                            nc.tensor.matmul(
                                pm[:, :], lhsT1[:, qsl], rhs1[:, sl],
                                start=True, stop=False,
                            )
                            nc.tensor.matmul(
                                pm[:, :], lhsT2[:, qsl], rhs2[:, sl],
                                start=False, stop=True,
                            )
                            nc.scalar.activation(
                                u[:, k * PCHUNK : (k + 1) * PCHUNK],
                                pm[:, :],
                                mybir.ActivationFunctionType.Relu,
                                bias=offb[:, 0:1],
                                scale=-ALPHA2,
                            )
                        nc.gpsimd.tensor_tensor(
                            u[:, :], u[:, :], n8u[:, :], mybir.AluOpType.add
                        )
                        for h in range(SPAN // SCHUNK):
                            gg = j * (SPAN // SCHUNK) + h
                            nc.vector.max(
                                cand[:, gg * 8 : (gg + 1) * 8],
                                u[:, h * SCHUNK : (h + 1) * SCHUNK].bitcast(F32),
                            )

                    v24 = cpool.tile([128, 24], F32, tag="v24")
                    p24 = cpool.tile([128, 24], U32, tag="p24")
                    for r in range(3):
                        rsl = slice(r * 8, (r + 1) * 8)
                        nc.vector.max(v24[:, rsl], cand[:, :])
                        nc.vector.max_index(p24[:, rsl], v24[:, rsl], cand[:, :])
                        if r < 2:
                            nc.vector.match_replace(
                                cand[:, :], v24[:, rsl], cand[:, :], NEG_BIG
                            )

                    nc.sync.dma_start(
                        outv.ap()[qsl, :], v24[:, 0:17:2].bitcast(I32)
                    )
                    nc.sync.dma_start(
                        outp.ap()[qsl, :], p24[:, 0:17:2].bitcast(I32)
                    )
    return nc


_COMPILED = None


def _get_compiled():
    global _COMPILED
    if _COMPILED is None:
        _install_ntff_shim()
        import concourse.bacc as bacc

        nc = bacc.Bacc("TRN2", target_bir_lowering=False, debug=False)
        build_kernel(nc)
        nc.compile()
        _COMPILED = nc
    return _COMPILED


LAST_RESULTS = None


def kernel(query: np.ndarray, _trace=False, _tmpdir=None) -> np.ndarray:
    global LAST_RESULTS
    from concourse import bass_utils

    query = np.ascontiguousarray(query, dtype=np.float32)
    assert query.shape == (B, N, C), query.shape
    nc = _get_compiled()

    in_maps = []
    for core in range(N_CORES):
        b, h = divmod(core, 2)
        in_maps.append(
            {
                "xq": query[b, h * NQ : (h + 1) * NQ, :],
                "xs": query[b],
            }
        )
    res = bass_utils.run_bass_kernel_spmd(
        nc, in_maps, core_ids=list(range(N_CORES)), trace=_trace, tmpdir=_tmpdir
    )
    LAST_RESULTS = res
    out = np.empty((B, N, K_OUT), np.int32)
    for core in range(N_CORES):
        b, h = divmod(core, 2)
        pv = res.results[core]["pkv"].view(np.uint32)
        pp = res.results[core]["pkp"].view(np.uint32)
        idx = (pp >> 3) * SCHUNK + (pv & (SCHUNK - 1))
        out[b, h * NQ : (h + 1) * NQ, :] = idx.astype(np.int32)
    return out


# revision 29
# speedup vs baseline: 1.0065x; 1.0065x over previous
"""Dilated KNN (k=9, dilation=2) over query[4, 8192, 64] on 8 NeuronCores.

Sharding: batch b and query-half h per core (core = 2*b + h). Each core
computes scores s[m, n] = 2*x_m.x_n - |x_n|^2 for its 4096 queries against
all 8192 supports of its batch, then selects the top-17 per row and emits
indices of ranks 0, 2, ..., 16.

Selection pipeline (single-scan, index-packed):
  PE  : fp32r hi/lo split matmul -> PSUM scores (exact products)
  ACT : u = uint32(Relu(-alpha2*s + 1.5*2^31)) -- the [2^31, 2^32) binade
        has ulp 256, so the cast's low 8 bits are zero for free; smaller u
        means better score
  POOL: p = u + n8 (n8 = column mod 256) -- packs the chunk-local index
        into the zeroed low byte; ties break toward the lowest index.
        A tunable fraction of spans packs on DVE (bitwise_or) instead to
        balance engine load.
  DVE : one MAX8 per 256-chunk on p bitcast to f32 (negative-float order
        reverses, so max8 finds the smallest u = best scores), then a
        3-round merge over the 256 candidates. Winner packed values and
        candidate positions DMA out; the host decodes
        idx = (pos>>3)*256 + (val&255). No second scan, no gathers.

Setup is kept off the ACT engine (DVE copies + DMA) so the main-loop
pipeline starts as early as possible; tile 0's query prep is emitted
before the support-side prep.
"""

import sys
import types

import numpy as np

B = 4
N = 8192
C = 64
K_OUT = 9
NQ = N // 2
N_CORES = 8
PCHUNK = 512              # psum matmul chunk (columns per bank)
SCHUNK = 256              # selection chunk (8-bit local index)
SPAN = 2048               # ACT/pool span (4 psum chunks, 8 sel chunks)
N_SPANS = N // SPAN       # 4
DVE_PACK_EVERY = 10 ** 9  # DVE int packs are slow/wrong; pool does all
ALPHA2 = 16000.0 * 256.0  # score quantization: delta = 256/ALPHA2 = 6.25e-5
OFFB = 1.5 * 2.0 ** 31
NEG_BIG = -1.0e38


def _install_ntff_shim():
    """bass_utils imports antenv.axon_hooks for trace=True; the agent image
    lacks it. Register the ctypes-based hook so NTFF profiling works."""
    if "antenv.axon_hooks" in sys.modules:
        return
    try:
        from trn_agent_boot.trn_boot import _ntff_profile_via_ctypes

        hook = _ntff_profile_via_ctypes("/opt/axon/libaxon_pjrt.so")
        m = types.ModuleType("antenv.axon_hooks")
        m.get_axon_ntff_profile_hook = lambda: hook
        sys.modules["antenv.axon_hooks"] = m
    except Exception:
        pass


def build_kernel(nc, n_queries=NQ):
    import concourse.mybir as mybir
    import concourse.tile as tile
    from concourse import masks

    F32 = mybir.dt.float32
    F32R = mybir.dt.float32r
    U32 = mybir.dt.uint32
    I32 = mybir.dt.int32

    m_tiles = n_queries // 128
    xq = nc.dram_tensor("xq", [n_queries, C], F32, kind="ExternalInput")
    xs = nc.dram_tensor("xs", [N, C], F32, kind="ExternalInput")
    # packed winner values and candidate positions; host decodes
    # idx = (pos >> 3) * 256 + (val & 255)
    outv = nc.dram_tensor("pkv", [n_queries, K_OUT], I32, kind="ExternalOutput")
    outp = nc.dram_tensor("pkp", [n_queries, K_OUT], I32, kind="ExternalOutput")
    sqh_d = nc.dram_tensor("sqh_d", [C, 128], F32, kind="Internal")
    sql_d = nc.dram_tensor("sql_d", [C, 128], F32, kind="Internal")

    with tile.TileContext(nc) as tc:
        with (
            tc.tile_pool(name="const", bufs=1) as constp,
            tc.tile_pool(name="big", bufs=1) as bigp,
        ):
            identity = constp.tile([128, 128], F32)
            masks.make_identity(nc, identity[:, :])
            ones2 = constp.tile([2, PCHUNK], F32)
            nc.vector.memset(ones2[:, :], 1.0)

            # n8 span constant: col mod 256, as uint32
            n8f = constp.tile([128, SPAN], F32)
            nc.gpsimd.iota(
                n8f[:, :],
                pattern=[[0, SPAN // SCHUNK], [1, SCHUNK]],
                base=0,
                channel_multiplier=0,
                allow_small_or_imprecise_dtypes=True,
            )
            n8u = constp.tile([128, SPAN], U32)
            nc.vector.tensor_copy(n8u[:, :], n8f[:, :])

            offb = constp.tile([128, 1], F32)
            nc.vector.memset(offb[:, :], OFFB)

            sqall = bigp.tile([128, C], F32)  # sq per support, [row, chunkblk]
            rhs1 = bigp.tile([128, N], F32R)
            rhs2 = bigp.tile([66, N], F32R)
            lhsT1 = bigp.tile([128, n_queries], F32R)
            lhsT2 = bigp.tile([66, n_queries], F32R)

            # lhsT2 ones rows don't depend on inputs
            nc.sync.dma_start(
                lhsT2[64:66, :].bitcast(F32).rearrange("p (r c) -> p r c", c=PCHUNK),
                ones2[:, :].unsqueeze(1).broadcast_to(
                    [2, n_queries // PCHUNK, PCHUNK]
                ),
            )

            with (
                tc.tile_pool(name="stage", bufs=4) as stagep,
                tc.tile_pool(name="dtmp", bufs=4) as dtmp,
                tc.tile_pool(name="ptr", bufs=4, space="PSUM") as ptrp,
                tc.tile_pool(name="psq", bufs=2, space="PSUM") as psqp,
            ):

                # query side first (engine queues drain these fast), so
                # the per-engine monotonic semaphores don't couple the main
                # loop to late setup ops; staged 4 tiles per DMA to keep the
                # sync queue short
                for q4 in range(m_tiles // 4):
                    st4 = stagep.tile([128, 4, C], F32, tag="stq")
                    nc.sync.dma_start(
                        st4[:, :, :],
                        xq.ap()[q4 * 512 : (q4 + 1) * 512, :].rearrange(
                            "(k p) c -> p k c", p=128
                        ),
                    )
                    for k in range(4):
                        t = q4 * 4 + k
                        qsl = slice(t * 128, (t + 1) * 128)
                        pt = ptrp.tile([C, 128], F32)
                        nc.tensor.transpose(
                            pt[:, :], st4[:, k, :], identity[:, :]
                        )
                        nc.scalar.mul(lhsT1[0:64, qsl], pt[:, :], 2.0)  # 2ah
                        # 2al = 2a - 2ah in one DVE op (F32R write rounds)
                        nc.vector.scalar_tensor_tensor(
                            lhsT1[64:128, qsl],
                            pt[:, :],
                            2.0,
                            lhsT1[0:64, qsl].bitcast(F32),
                            mybir.AluOpType.mult,
                            mybir.AluOpType.subtract,
                        )
                nc.sync.dma_start(
                    lhsT2[0:64, :].bitcast(F32), lhsT1[0:64, :].bitcast(F32)
                )

                # support side
                for cc in range(N // PCHUNK):
                    sl = slice(cc * PCHUNK, (cc + 1) * PCHUNK)
                    sqcol = sqall[:, cc * 4 : (cc + 1) * 4]
                    st4 = stagep.tile([128, 4, C], F32, tag="sts")
                    nc.sync.dma_start(
                        st4[:, :, :],
                        xs.ap()[sl, :].rearrange("(k p) c -> p k c", p=128),
                    )
                    sqscr = dtmp.tile([128, 4 * C], F32, tag="sqscr")
                    # |x_n|^2 per support row while it's still [n, c]
                    # (tensor_tensor_reduce hangs TRN2 here; use mul+reduce)
                    nc.vector.tensor_mul(
                        sqscr[:, :].rearrange("p (k c) -> p k c", c=C),
                        st4[:, :, :],
                        st4[:, :, :],
                    )
                    nc.vector.reduce_sum(
                        sqcol,
                        sqscr[:, :].rearrange("p (k c) -> p k c", c=C),
                        axis=mybir.AxisListType.X,
                    )
                    for k in range(PCHUNK // 128):
                        j = cc * (PCHUNK // 128) + k
                        jsl = slice(j * 128, (j + 1) * 128)
                        pt = ptrp.tile([C, 128], F32)
                        nc.tensor.transpose(
                            pt[:, :], st4[:, k, :], identity[:, :]
                        )
                        if k % 2 == 0:
                            nc.scalar.copy(rhs1[0:64, jsl], pt[:, :])  # bh
                        else:
                            nc.vector.tensor_copy(rhs1[0:64, jsl], pt[:, :])
                        nc.vector.tensor_sub(
                            rhs2[0:64, jsl], pt[:, :],
                            rhs1[0:64, jsl].bitcast(F32),
                        )  # bl
                    ptq = psqp.tile([PCHUNK // 128, 128], F32)
                    nc.tensor.transpose(ptq[:, :], sqcol, identity[:, :])
                    sq4 = dtmp.tile([PCHUNK // 128, 128], F32, tag="sq4")
                    nc.scalar.copy(sq4[:, :], ptq[:, :])
                    sqr = dtmp.tile([1, PCHUNK], F32, tag="sqr")
                    for k in range(PCHUNK // 128):
                        nc.sync.dma_start(
                            sqr[0:1, k * 128 : (k + 1) * 128], sq4[k : k + 1, :]
                        )
                    nsqh = dtmp.tile([1, PCHUNK], F32R, tag="nsqh")
                    nc.scalar.mul(nsqh[:, :], sqr[:, :], -1.0)  # -sqh
                    nc.sync.dma_start(rhs2[64:65, sl], nsqh[:, :])
                    sql = dtmp.tile([1, PCHUNK], F32, tag="sql")
                    nc.vector.tensor_add(sql[:, :], sqr[:, :], nsqh[:, :].bitcast(F32))
                    nsql = dtmp.tile([1, PCHUNK], F32R, tag="nsql")
                    nc.scalar.mul(nsql[:, :], sql[:, :], -1.0)  # -sql
                    nc.sync.dma_start(rhs2[65:66, sl], nsql[:, :])
                    nc.sync.dma_start(
                        rhs1[64:128, sl].bitcast(F32), rhs1[0:64, sl].bitcast(F32)
                    )

            with (
                tc.tile_pool(name="upool", bufs=3) as upool,
                tc.tile_pool(name="ppool", bufs=3) as ppool,
                tc.tile_pool(name="cpool", bufs=2) as cpool,
                tc.tile_pool(name="pmm", bufs=8, space="PSUM") as pmm,
            ):
                for t in range(m_tiles):
                    qsl = slice(t * 128, (t + 1) * 128)
                    cand = cpool.tile([128, 256], F32, tag="cand")
                    for j in range(N_SPANS):
                        u = upool.tile([128, SPAN], U32, tag="u")
                        for k in range(SPAN // PCHUNK):
                            cc = j * (SPAN // PCHUNK) + k
                            sl = slice(cc * PCHUNK, (cc + 1) * PCHUNK)
                            pm = pmm.tile([128, PCHUNK], F32, tag="pm")
                            nc.tensor.matmul(
                                pm[:, :], lhsT1[:, qsl], rhs1[:, sl],
                                start=True, stop=False,
                            )
                            nc.tensor.matmul(
                                pm[:, :], lhsT2[:, qsl], rhs2[:, sl],
                                start=False, stop=True,
                            )
                            nc.scalar.activation(
                                u[:, k * PCHUNK : (k + 1) * PCHUNK],
                                pm[:, :],
                                mybir.ActivationFunctionType.Relu,
                                bias=offb[:, 0:1],
                                scale=-ALPHA2,
                            )
                        p = ppool.tile([128, SPAN], U32, tag="p")
                        g = t * N_SPANS + j
                        if g % DVE_PACK_EVERY == DVE_PACK_EVERY - 1:
                            nc.vector.tensor_tensor(
                                p[:, :], u[:, :], n8u[:, :],
                                mybir.AluOpType.bitwise_or,
                            )
                        else:
                            nc.gpsimd.tensor_tensor(
                                p[:, :], u[:, :], n8u[:, :], mybir.AluOpType.add
                            )
                        for h in range(SPAN // SCHUNK):
                            gg = j * (SPAN // SCHUNK) + h
                            nc.vector.max(
                                cand[:, gg * 8 : (gg + 1) * 8],
                                p[:, h * SCHUNK : (h + 1) * SCHUNK].bitcast(F32),
                            )

                    v24 = cpool.tile([128, 24], F32, tag="v24")
                    p24 = cpool.tile([128, 24], U32, tag="p24")
                    for r in range(3):
                        rsl = slice(r * 8, (r + 1) * 8)
                        nc.vector.max(v24[:, rsl], cand[:, :])
                        nc.vector.max_index(p24[:, rsl], v24[:, rsl], cand[:, :])
                        if r < 2:
                            nc.vector.match_replace(
                                cand[:, :], v24[:, rsl], cand[:, :], NEG_BIG
                            )

                    nc.sync.dma_start(
                        outv.ap()[qsl, :], v24[:, 0:17:2].bitcast(I32)
                    )
                    nc.sync.dma_start(
                        outp.ap()[qsl, :], p24[:, 0:17:2].bitcast(I32)
                    )
    return nc


_COMPILED = None


def _get_compiled():
    global _COMPILED
    if _COMPILED is None:
        _install_ntff_shim()
        import concourse.bacc as bacc

        nc = bacc.Bacc("TRN2", target_bir_lowering=False, debug=False)
        build_kernel(nc)
        nc.compile()
        _COMPILED = nc
    return _COMPILED


LAST_RESULTS = None


def kernel(query: np.ndarray, _trace=False, _tmpdir=None) -> np.ndarray:
    global LAST_RESULTS
    from concourse import bass_utils

    query = np.ascontiguousarray(query, dtype=np.float32)
    assert query.shape == (B, N, C), query.shape
    nc = _get_compiled()

    in_maps = []
    for core in range(N_CORES):
        b, h = divmod(core, 2)
        in_maps.append(
            {
                "xq": query[b, h * NQ : (h + 1) * NQ, :],
                "xs": query[b],
            }
        )
    res = bass_utils.run_bass_kernel_spmd(
        nc, in_maps, core_ids=list(range(N_CORES)), trace=_trace, tmpdir=_tmpdir
    )
    LAST_RESULTS = res
    out = np.empty((B, N, K_OUT), np.int32)
    for core in range(N_CORES):
        b, h = divmod(core, 2)
        pv = res.results[core]["pkv"].view(np.uint32)
        pp = res.results[core]["pkp"].view(np.uint32)
        idx = (pp >> 3) * SCHUNK + (pv & (SCHUNK - 1))
        out[b, h * NQ : (h + 1) * NQ, :] = idx.astype(np.int32)
    return out
